# revision 7
# baseline (speedup 1.0000x reference)
"""Chebyshev Graph ConvNet (LeNet5-style GCN) on 8 Trainium2 NeuronCores.

One SPMD launch computes the full graph-conv front-end on device (core m
handles batches 8m..8m+7): GC1 = K=25 Chebyshev SpMM at width 64 on the
16384-node graph (replicated across cores), combine/relu/pool; GC2 = K=25
SpMM at width 256 (8 batches x 32 features, batch-sharded), combine/relu/
pool -> h [8, 64, 1024] per core. SpMM = dma_gather of Chebyshev-plane rows
plus staircase scatter matmuls on the PE with device-expanded S chunks.
FC1/FC2 run on host BLAS (uploading the 134MB fc1_W over the transfer link
would dominate wall time; the host matmul takes ~70ms).
"""
import sys
sys.path.insert(0, "/opt/trn_rl_repo")
import numpy as np
import ml_dtypes
import concourse.bacc as bacc
import concourse.bass as bass
import concourse.mybir as mybir
import concourse.tile as tile
from concourse.bass import ds
from concourse.library_config import mlp
from concourse.masks import make_identity

f32 = mybir.dt.float32
f32r = mybir.dt.float32r
bf16 = mybir.dt.bfloat16
i16 = mybir.dt.int16
u8 = mybir.dt.uint8

P = 128
WIN = 128
WPT = 1


class Cfg:
    def __init__(self, V1=16384, V2=4096, CPW1=6, CPW2=6, BLOC=8, K=25, unroll=4):
        self.V1, self.V2 = V1, V2
        self.T1, self.T2 = V1 // P, V2 // P
        self.CPW1, self.CPW2 = CPW1, CPW2
        self.CPT1, self.CPT2 = CPW1 * WPT, CPW2 * WPT
        self.BLOC = BLOC
        self.K = K
        self.W1 = 64
        self.W2 = BLOC * 32
        self.V3 = V2 // 4
        self.unroll = unroll
        self.KG = (K + 3) // 4


def build_program(c: Cfg):
    nc = bacc.Bacc("TRN2", target_bir_lowering=False, debug=False,
                   enable_partition_id=True)
    NC1 = c.T1 * c.CPT1
    NC2 = c.T2 * c.CPT2

    xT = nc.dram_tensor("xT", [c.V1, c.W1], bf16, kind="ExternalInput")
    g1_idx = nc.dram_tensor("g1_idx", [16, NC1 * 8], i16, kind="ExternalInput")
    g1_off = nc.dram_tensor("g1_off", [P, NC1], u8, kind="ExternalInput")
    g1_val = nc.dram_tensor("g1_val", [P, NC1], bf16, kind="ExternalInput")
    g2_idx = nc.dram_tensor("g2_idx", [16, NC2 * 8], i16, kind="ExternalInput")
    g2_off = nc.dram_tensor("g2_off", [P, NC2], u8, kind="ExternalInput")
    g2_val = nc.dram_tensor("g2_val", [P, NC2], bf16, kind="ExternalInput")
    w1T = nc.dram_tensor("w1T", [c.K, 32], bf16, kind="ExternalInput")
    b1v = nc.dram_tensor("b1v", [32, 1], f32, kind="ExternalInput")
    w2kg = nc.dram_tensor("w2kg", [P, c.KG, 64], f32, kind="ExternalInput")
    b2v = nc.dram_tensor("b2v", [64, 1], f32, kind="ExternalInput")
    iota = nc.dram_tensor("iota", [P, WIN], f32, kind="ExternalInput")
    hout = nc.dram_tensor("hout", [c.BLOC, 64, c.V3], f32, kind="ExternalOutput")

    with tile.TileContext(nc) as tc:
        with (
            tc.tile_pool(name="drp", bufs=1, space="DRAM") as drp,
            tc.tile_pool(name="cst", bufs=1) as cst,
        ):
            X1 = drp.tile([c.K, c.V1, c.W1], f32)
            X1S = drp.tile([c.K, c.V1, c.BLOC], f32)
            X2 = drp.tile([c.K, c.V2, c.W2], f32)
            X2T = drp.tile([c.K, c.W2, c.V2], f32)
            S1 = drp.tile([P, c.T1, c.CPT1, WIN], f32)
            S2 = drp.tile([P, c.T2, c.CPT2, WIN], f32)

            nc.gpsimd.load_library(mlp)
            ident = cst.tile([P, P], f32)
            make_identity(nc, ident[:])
            iota_sb = cst.tile([P, WIN], f32)
            nc.sync.dma_start(out=iota_sb[:], in_=iota[:])
            w1T_sb = cst.tile([c.K, 32], bf16)
            nc.sync.dma_start(out=w1T_sb[:], in_=w1T[:])
            b1_sb = cst.tile([32, 1], f32)
            nc.sync.dma_start(out=b1_sb[:], in_=b1v[:])
            b2_sb = cst.tile([64, 1], f32)
            nc.sync.dma_start(out=b2_sb[:], in_=b2v[:])
            w2_sb = cst.tile([P, c.KG, 64], f32r)
            nc.sync.dma_start(out=w2_sb[:], in_=w2kg[:].bitcast(f32r))

            idx1_sb = cst.tile([P, NC1 * 8], i16)
            idx2_sb = cst.tile([P, NC2 * 8], i16)
            for g in range(8):
                nc.sync.dma_start(out=idx1_sb[g * 16:(g + 1) * 16, :], in_=g1_idx[:])
                nc.sync.dma_start(out=idx2_sb[g * 16:(g + 1) * 16, :], in_=g2_idx[:])

            # ---- S expansion ----
            def expand_S(S, offs, vals, T, CPT, pool):
                BB = 8 * CPT
                nchunk = T * CPT
                for b0 in range(0, nchunk, BB):
                    bb = min(BB, nchunk - b0)
                    off_sb = pool.tile([P, BB], u8, tag="off8")
                    val_sb = pool.tile([P, BB], bf16, tag="val8")
                    nc.sync.dma_start(out=off_sb[:, :bb], in_=offs[:, b0:b0 + bb])
                    nc.sync.dma_start(out=val_sb[:, :bb], in_=vals[:, b0:b0 + bb])
                    off_f = pool.tile([P, BB], f32, tag="offf")
                    val_f = pool.tile([P, BB], f32, tag="valf")
                    nc.vector.tensor_copy(out=off_f[:, :bb], in_=off_sb[:, :bb])
                    nc.vector.tensor_copy(out=val_f[:, :bb], in_=val_sb[:, :bb])
                    sexp = pool.tile([P, BB, WIN], f32, tag="sexp")
                    nc.vector.tensor_tensor(
                        out=sexp[:, :bb, :],
                        in0=off_f[:, :bb].rearrange("p (b o) -> p b o", o=1).to_broadcast([P, bb, WIN]),
                        in1=iota_sb[:].rearrange("(p o) w -> p o w", o=1).to_broadcast([P, bb, WIN]),
                        op=mybir.AluOpType.is_equal,
                    )
                    nc.vector.tensor_tensor(
                        out=sexp[:, :bb, :],
                        in0=sexp[:, :bb, :],
                        in1=val_f[:, :bb].rearrange("p (b o) -> p b o", o=1).to_broadcast([P, bb, WIN]),
                        op=mybir.AluOpType.mult,
                    )
                    nc.sync.dma_start(
                        out=S[:].rearrange("p t c w -> p (t c) w")[:, b0:b0 + bb, :],
                        in_=sexp[:, :bb, :],
                    )

            with tc.tile_pool(name="sexp", bufs=2) as sp:
                expand_S(S1, g1_off, g1_val, c.T1, c.CPT1, sp)
                expand_S(S2, g2_off, g2_val, c.T2, c.CPT2, sp)

            # ---- init plane 0 of X1 ----
            with tc.tile_pool(name="init", bufs=2) as ip:
                for t in range(c.T1):
                    xt = ip.tile([P, c.W1], bf16, tag="xt")
                    nc.sync.dma_start(out=xt[:], in_=xT[t * P:(t + 1) * P, :])
                    xf = ip.tile([P, c.W1], f32, tag="xf")
                    nc.vector.tensor_copy(out=xf[:], in_=xt[:])
                    nc.sync.dma_start(out=X1[0, t * P:(t + 1) * P, :], in_=xf[:])

            # ---- spmm iterations ----
            def spmm_layer(X, S, idx_sb, T, CPT, W, tag):
                CW = CPT // WPT
                for k in range(1, c.K):
                    first = k == 1
                    with (
                        tc.tile_pool(name=f"i{tag}{k}", bufs=3) as pl,
                        tc.tile_pool(name=f"p{tag}{k}", bufs=4, space="PSUM") as pp,
                    ):
                        def body(t):
                            g = pl.tile([P, CPT, W], f32r, tag="g")
                            for c0 in range(0, CPT, 8):
                                cc = min(8, CPT - c0)
                                nc.gpsimd.dma_gather(
                                    g[:, c0:c0 + cc, :], X[k - 1].bitcast(f32r),
                                    idx_sb[:, ds(t * (CPT * 8) + c0 * 8, cc * 8)],
                                    cc * P, cc * P, W,
                                )
                            s = pl.tile([P, CPT, WIN], f32r, tag="s")
                            nc.sync.dma_start(
                                out=s[:],
                                in_=S[:, ds(t, 1)].bitcast(f32r)
                                    .rearrange("p o c w -> p (o c) w"),
                            )
                            ps = pp.tile([P, W], f32, tag="ps")
                            for cc in range(CPT):
                                w = cc // CW
                                nc.tensor.matmul(
                                    out=ps[w * WIN:(w + 1) * WIN, :],
                                    lhsT=s[:, cc, :],
                                    rhs=g[:, cc, :],
                                    start=(cc % CW == 0),
                                    stop=(cc % CW == CW - 1),
                                )
                            ym1 = pl.tile([P, W], f32, tag="ym1")
                            nc.sync.dma_start(out=ym1[:], in_=X[k - 1, ds(t * P, P), :])
                            yn = pl.tile([P, W], f32, tag="yn")
                            nc.vector.tensor_tensor(
                                out=yn[:], in0=ps[:], in1=ym1[:],
                                op=mybir.AluOpType.subtract,
                            )
                            if not first:
                                ym2 = pl.tile([P, W], f32, tag="ym2")
                                nc.sync.dma_start(
                                    out=ym2[:], in_=X[k - 2, ds(t * P, P), :]
                                )
                                nc.vector.tensor_scalar_mul(yn[:], yn[:], 2.0)
                                nc.vector.tensor_tensor(
                                    out=yn[:], in0=yn[:], in1=ym2[:],
                                    op=mybir.AluOpType.subtract,
                                )
                            nc.sync.dma_start(out=X[k, ds(t * P, P), :], in_=yn[:])

                        tc.For_i_unrolled(0, T, 1, body, max_unroll=c.unroll)

            spmm_layer(X1, S1, idx1_sb, c.T1, c.CPT1, c.W1, "a")

            # ---- slice out this core's 8 columns of X1 ----
            cb = nc.sync.partition_id() * c.BLOC
            with tc.tile_pool(name="slc", bufs=2) as slp:
                for k in range(c.K):
                    sl = slp.tile([P, c.V1 // P, c.BLOC], f32, tag="sl")
                    nc.sync.dma_start(
                        out=sl[:],
                        in_=X1[k].rearrange("(t p) w -> p t w", p=P)[:, :, ds(cb, c.BLOC)],
                    )
                    nc.sync.dma_start(
                        out=X1S[k].rearrange("(t p) w -> p t w", p=P), in_=sl[:]
                    )

            # ---- GC1 combine + pool -> X2 plane 0 ----
            NCH = c.V1 * c.BLOC // 512
            with (
                tc.tile_pool(name="cb1", bufs=3) as pl,
                tc.tile_pool(name="cp1", bufs=4, space="PSUM") as pp,
            ):
                def body(ch):
                    rhs = pl.tile([c.K, 512], bf16, tag="rhs")
                    nc.gpsimd.dma_start(
                        out=rhs[:].rearrange("k (v b) -> k v b", b=c.BLOC),
                        in_=X1S[:, ds(ch * 64, 64), :],
                    )
                    ps = pp.tile([32, 512], f32, tag="cps1")
                    nc.tensor.matmul(out=ps[:], lhsT=w1T_sb[:], rhs=rhs[:],
                                     start=True, stop=True)
                    act = pl.tile([32, 512], f32, tag="act")
                    nc.vector.tensor_tensor(
                        out=act[:], in0=ps[:],
                        in1=b1_sb[:].to_broadcast([32, 512]),
                        op=mybir.AluOpType.add,
                    )
                    nc.vector.tensor_scalar_max(act[:], act[:], 0.0)
                    poo = pl.tile([32, P], f32, tag="poo")
                    a4 = act[:].rearrange("f (g d b) -> f g d b", d=4, b=c.BLOC)
                    p4 = poo[:].rearrange("f (g b) -> f g b", b=c.BLOC)
                    nc.vector.tensor_tensor(out=p4, in0=a4[:, :, 0, :],
                                            in1=a4[:, :, 1, :], op=mybir.AluOpType.max)
                    nc.vector.tensor_tensor(out=p4, in0=p4, in1=a4[:, :, 2, :],
                                            op=mybir.AluOpType.max)
                    nc.vector.tensor_tensor(out=p4, in0=p4, in1=a4[:, :, 3, :],
                                            op=mybir.AluOpType.max)
                    tp = pp.tile([P, 32], f32, tag="tps")
                    nc.tensor.transpose(out=tp[:], in_=poo[:], identity=ident[:32, :32])
                    tps = pl.tile([P, 32], f32, tag="tpsb")
                    nc.vector.tensor_copy(out=tps[:], in_=tp[:])
                    nc.sync.dma_start(
                        out=X2[0, ds(ch * 16, 16), :]
                            .rearrange("v c -> (v c)").rearrange("(p f) -> p f", p=P),
                        in_=tps[:],
                    )

                tc.For_i_unrolled(0, NCH, 1, body, max_unroll=c.unroll)

            # ---- GC2 spmm ----
            spmm_layer(X2, S2, idx2_sb, c.T2, c.CPT2, c.W2, "b")

            # ---- GC2 combine phase A: planes -> X2T ----
            with (
                tc.tile_pool(name="trA", bufs=3) as pl,
                tc.tile_pool(name="tAp", bufs=4, space="PSUM") as pp,
            ):
                for k in range(c.K):
                    for h in range(c.W2 // P):
                        def body(t):
                            src = pl.tile([P, P], f32, tag="tsrc")
                            nc.sync.dma_start(
                                out=src[:],
                                in_=X2[k, ds(t * P, P), h * P:(h + 1) * P],
                            )
                            tp = pp.tile([P, P], f32, tag="tpp")
                            nc.tensor.transpose(out=tp[:], in_=src[:],
                                                identity=ident[:])
                            dst = pl.tile([P, P], f32, tag="tdst")
                            nc.vector.tensor_copy(out=dst[:], in_=tp[:])
                            nc.sync.dma_start(
                                out=X2T[k, h * P:(h + 1) * P, ds(t * P, P)],
                                in_=dst[:],
                            )
                        tc.For_i_unrolled(0, c.T2, 1, body, max_unroll=c.unroll)

            # ---- GC2 combine phase B ----
            with (
                tc.tile_pool(name="cb2", bufs=3) as pl,
                tc.tile_pool(name="cp2", bufs=4, space="PSUM") as pp,
            ):
                NV = c.V2 // 512
                for b in range(c.BLOC):
                    def body(vc):
                        ps = pp.tile([64, 512], f32, tag="ps2")
                        for g in range(c.KG):
                            rhs = pl.tile([P, 512], f32r, tag="rhs2")
                            for j in range(4):
                                kplane = g * 4 + j if g * 4 + j < c.K else 0
                                nc.sync.dma_start(
                                    out=rhs[j * 32:(j + 1) * 32, :],
                                    in_=X2T[kplane, b * 32:(b + 1) * 32,
                                            ds(vc * 512, 512)].bitcast(f32r),
                                )
                            nc.tensor.matmul(
                                out=ps[:], lhsT=w2_sb[:, g, :],
                                rhs=rhs[:],
                                start=(g == 0), stop=(g == c.KG - 1),
                            )
                        act = pl.tile([64, 512], f32, tag="act2")
                        nc.vector.tensor_tensor(
                            out=act[:], in0=ps[:],
                            in1=b2_sb[:].to_broadcast([64, 512]),
                            op=mybir.AluOpType.add,
                        )
                        nc.vector.tensor_scalar_max(act[:], act[:], 0.0)
                        poo = pl.tile([64, P], f32, tag="poo2")
                        a4 = act[:].rearrange("f (g d) -> f g d", d=4)
                        nc.vector.tensor_tensor(out=poo[:], in0=a4[:, :, 0],
                                                in1=a4[:, :, 1], op=mybir.AluOpType.max)
                        nc.vector.tensor_tensor(out=poo[:], in0=poo[:], in1=a4[:, :, 2],
                                                op=mybir.AluOpType.max)
                        nc.vector.tensor_tensor(out=poo[:], in0=poo[:], in1=a4[:, :, 3],
                                                op=mybir.AluOpType.max)
                        nc.sync.dma_start(out=hout[b, :, ds(vc * P, P)], in_=poo[:])
                    tc.For_i_unrolled(0, NV, 1, body, max_unroll=2)

    nc.compile()
    return nc


# ---------------- host packer ----------------

def pack_sparse(rows, cols, vals, V, CPW):
    T = V // P
    CPT = CPW * WPT
    NC = T * CPT
    nw = V // WIN
    w = (rows // WIN).astype(np.int64)
    order = np.argsort(w, kind="stable")
    rows, cols, vals = rows[order], cols[order], vals[order]
    w = w[order]
    counts = np.bincount(w, minlength=nw)
    if counts.max() > CPW * P:
        raise ValueError(f"window overflow: {counts.max()} > {CPW * P}")
    starts = np.zeros(nw + 1, np.int64)
    np.cumsum(counts, out=starts[1:])
    rank = np.arange(len(rows)) - starts[w]
    chunk = w * CPW + rank // P
    part = rank % P
    idx = np.zeros((NC, P), np.int16)
    off = np.zeros((NC, P), np.uint8)
    val = np.zeros((NC, P), np.float32)
    idx[chunk, part] = cols.astype(np.int16)
    off[chunk, part] = (rows % WIN).astype(np.uint8)
    val[chunk, part] = vals
    wrapped = idx.reshape(-1).reshape(-1, 16).T.copy()       # [16, NC*8]
    return wrapped, off.T.copy(), val.T.astype(ml_dtypes.bfloat16).copy()


def make_w2kg(cl2_W, K, KG):
    # cl2_W [64, 32*K] with column = fin*K + k -> w2kg [128, KG, 64]
    w = cl2_W.reshape(64, 32, K)
    out = np.zeros((P, KG, 64), np.float32)
    for g in range(KG):
        kk = min(4, K - g * 4)
        for j in range(kk):
            # lhsT rows = j*32 + fin, cols = fout
            out[j * 32:(j + 1) * 32, g, :] = w[:, :, g * 4 + j].T
    return out


def make_host_inputs(inp, c: Cfg, core):
    """Per-core input dict for run_bass_kernel_spmd."""
    out = {}
    out["xT"] = np.ascontiguousarray(
        inp["x"].astype(np.float32).T).astype(ml_dtypes.bfloat16)
    i1, o1, v1 = pack_sparse(np.asarray(inp["L0_rows"]), np.asarray(inp["L0_cols"]),
                             np.asarray(inp["L0_vals"], np.float32), c.V1, c.CPW1)
    out["g1_idx"], out["g1_off"], out["g1_val"] = i1, o1, v1
    i2, o2, v2 = pack_sparse(np.asarray(inp["L2_rows"]), np.asarray(inp["L2_cols"]),
                             np.asarray(inp["L2_vals"], np.float32), c.V2, c.CPW2)
    out["g2_idx"], out["g2_off"], out["g2_val"] = i2, o2, v2
    out["w1T"] = np.ascontiguousarray(
        np.asarray(inp["cl1_W"], np.float32).T).astype(ml_dtypes.bfloat16)
    out["b1v"] = np.asarray(inp["cl1_b"], np.float32).reshape(32, 1)
    out["w2kg"] = make_w2kg(np.asarray(inp["cl2_W"], np.float32), c.K, c.KG)
    out["b2v"] = np.asarray(inp["cl2_b"], np.float32).reshape(64, 1)
    out["iota"] = np.tile(np.arange(WIN, dtype=np.float32)[None, :], (P, 1))
    return out


# ======================= entry point =======================

N_CORES = 8
B = 64

_PROG = None
_CFG = None
LAST_HW_EXEC_NS = None


def _get_prog():
    global _PROG, _CFG
    if _PROG is None:
        _CFG = Cfg(V1=16384, V2=4096, CPW1=18, CPW2=18, BLOC=8, K=25, unroll=4)
        _PROG = build_program(_CFG)
    return _PROG, _CFG


def kernel(x, L0_rows, L0_cols, L0_vals, L2_rows, L2_cols, L2_vals,
           cl1_W, cl1_b, cl2_W, cl2_b, fc1_W, fc1_b, fc2_W, fc2_b):
    from concourse.bass_utils import run_bass_kernel_spmd

    nc, c = _get_prog()
    inp = {"x": x, "L0_rows": L0_rows, "L0_cols": L0_cols, "L0_vals": L0_vals,
           "L2_rows": L2_rows, "L2_cols": L2_cols, "L2_vals": L2_vals,
           "cl1_W": cl1_W, "cl1_b": cl1_b, "cl2_W": cl2_W, "cl2_b": cl2_b}
    hin = make_host_inputs(inp, c, 0)
    in_maps = [hin for _ in range(N_CORES)]
    res = run_bass_kernel_spmd(nc, in_maps, core_ids=list(range(N_CORES)))

    h2 = np.empty((B, 65536), np.float32)
    for m in range(N_CORES):
        hm = np.asarray(res.results[m]["hout"])  # [8, 64, 1024]
        for i in range(8):
            h2[m * 8 + i] = hm[i].T.reshape(-1)

    fc1_W = np.asarray(fc1_W, np.float32)
    h = np.maximum(h2 @ fc1_W.T + np.asarray(fc1_b, np.float32), 0.0)
    return h @ np.asarray(fc2_W, np.float32).T + np.asarray(fc2_b, np.float32)
